# revision 40
# baseline (speedup 1.0000x reference)
"""Trainium2 Bass kernel for HNet attention (B=4, S=2048, H=768, 12 heads, RoPE, causal).

Sharding: 8 cores = 4 batches x 2 head-groups (6 heads each).
Wq/Wk/Wv split column-wise (head axis), Wo row-wise; host sums the two
partial o_proj outputs per batch (the "all-reduce" done at gather time).

Per-core dataflow (all matmuls in float32r = 1 cyc/row on the PE):
  xT [768,2048] (host-transposed) --PE--> Q,K,V natural [2048,384]
  RoPE on Q,K in natural layout (DVE/GPSIMD), PE-transpose -> QT,KT [384,2048]
  scoresT[k,q] = KT_tile.T @ QT  (per head, causal-trimmed strips)
  exp on ScalarE (no max subtraction; scores ~ N(0,1)), diag tiles masked
  PV: lhsT = [V_h | ones] [k,65] -> attn_outT rows 0:64 + softmax sums row 64
  sums -> (SBUF->SBUF DMA gather) -> reciprocal -> K=6 broadcast matmul -> scale
  o_proj: lhsT = attn_outT tiles, rhs = WoT -> out [2048,768] partial
"""

import os
import sys

import numpy as np

sys.path.insert(0, "/opt/trn_rl_repo")

from contextlib import ExitStack

import concourse.bacc as bacc
import concourse.tile as tile
from concourse import mybir
from concourse.bass_utils import run_bass_kernel_spmd

S = 2048
HID = 768
NH = 6            # heads per core
D = 64
F = NH * D        # 384 per-core feature slice
P = 128
SC = S // P       # 16
FC = HID // P     # 6
MC = F // P       # 3
QW = 512          # q strip width
NQ = S // QW      # 4
N_CORES = 8
ROPE_THETA = 10000.0

F32 = mybir.dt.float32
F32R = mybir.dt.float32r
FP8 = mybir.dt.float8e4
BF16 = mybir.dt.bfloat16
DR = mybir.MatmulPerfMode.DoubleRow
AF = mybir.ActivationFunctionType
EXP_BIAS = -2.0  # exp(s/8 - 2): keeps ex in fp8 range; cancels in softmax


def _r(ap):
    """tiles are allocated as float32r already; keep as passthrough."""
    return ap


def _h3(ap):
    """[P, F] -> [P, NH, D] view."""
    return ap.rearrange("p (h d) -> p h d", h=NH)


def build_program():
    nc = bacc.Bacc("TRN2", target_bir_lowering=False, debug=False,
                   num_devices=N_CORES)

    xT_d = nc.dram_tensor("xT", [HID, S], BF16, kind="ExternalInput").ap()
    wqT_d = nc.dram_tensor("wqT", [HID, F], BF16, kind="ExternalInput").ap()
    wkT_d = nc.dram_tensor("wkT", [HID, F], BF16, kind="ExternalInput").ap()
    wvT_d = nc.dram_tensor("wvT", [HID, F], BF16, kind="ExternalInput").ap()
    woT_d = nc.dram_tensor("woT", [F, HID], BF16, kind="ExternalInput").ap()
    cos_d = nc.dram_tensor("cos6", [S, F], BF16, kind="ExternalInput").ap()
    sin_d = nc.dram_tensor("sin6", [S, F], BF16, kind="ExternalInput").ap()
    cb_d = nc.dram_tensor("cblob", [P, 3 * P + NH], BF16,
                          kind="ExternalInput").ap()
    e_d = nc.dram_tensor("emat", [NH, F], F32R, kind="ExternalInput").ap()
    out_d = nc.dram_tensor("out", [S, HID], BF16, kind="ExternalOutput").ap()

    with tile.TileContext(nc) as tc, ExitStack() as ctx:
        const_pool = ctx.enter_context(tc.tile_pool(name="const", bufs=1))
        cb_sb = const_pool.tile([P, 3 * P + NH], BF16, tag="cblob")
        nc.sync.dma_start(cb_sb[:], cb_d[:])
        tri_sb = cb_sb[:, 0:2 * P].rearrange("p (o k) -> p o k", o=2)
        eye_sb = cb_sb[:, 2 * P:3 * P]
        on_sb = cb_sb[:, 3 * P:3 * P + NH]
        e_sb = const_pool.tile([NH, F], F32R, tag="emat")
        nc.sync.dma_start(e_sb[:], e_d[:])
        ebias_sb = const_pool.tile([P, 1], F32, tag="ebias")
        nc.gpsimd.memset(ebias_sb[:], EXP_BIAS)

        # persistent per-phase tensors
        qkT_pool = ctx.enter_context(tc.tile_pool(name="qkT", bufs=1))
        kT = qkT_pool.tile([P, MC, S], BF16, tag="kT", name="kT")
        v_pool = ctx.enter_context(tc.tile_pool(name="vp", bufs=1))
        v_sb = [v_pool.tile([P, NH * 65], BF16, tag=f"v{s}", name=f"v{s}") for s in range(SC)]
        for s in range(SC):
            nc.vector.tensor_copy(
                v_sb[s].rearrange("p (h e) -> p h e", h=NH)[:, :, 64:65],
                on_sb.rearrange("p (h o) -> p h o", h=NH))
        ao_pool = ctx.enter_context(tc.tile_pool(name="ao", bufs=2))
        woT_pool = ctx.enter_context(tc.tile_pool(name="woT", bufs=1))
        woT = woT_pool.tile([P, MC, HID], BF16, tag="woT", name="woT")
        nc.sync.dma_start(woT[:], woT_d.rearrange("(m p) c -> p m c", p=P))

        # ---- single interleaved phase: proj-group(qc) then attention(qc) ----
        # PSUM banks (8): pq1 + pk1 + ring3 (pv/pt/sc/bp) + pvp1 + fin2
        with tc.tile_pool(name="xT", bufs=1) as xT_pool, \
             tc.tile_pool(name="wT", bufs=1) as wT_pool, \
             tc.tile_pool(name="ld", bufs=2) as ld_pool, \
             tc.tile_pool(name="rope", bufs=2) as rope_pool, \
             tc.tile_pool(name="ex", bufs=9) as ex_pool, \
             tc.tile_pool(name="stg", bufs=2) as stg_pool, \
             tc.tile_pool(name="sums", bufs=2) as sums_pool, \
             tc.tile_pool(name="ob", bufs=2) as ob_pool, \
             tc.tile_pool(name="ps_qk", bufs=1, space="PSUM") as ps_qk, \
             tc.tile_pool(name="ring", bufs=3, space="PSUM") as ring, \
             tc.tile_pool(name="ps_pv", bufs=2, space="PSUM") as ps_pv:

            cs_all = xT_pool.tile([P, SC * F], BF16, tag="cosall", name="cs_all")
            sn_all = xT_pool.tile([P, SC * F], BF16, tag="sinall", name="sn_all")

            def _trig_chunk(c, w):
                ts_ = slice(c * 4, (c + w) * 4)
                fs_ = slice(c * 4 * F, (c + w) * 4 * F)
                nc.sync.dma_start(
                    cs_all[:, fs_].rearrange("p (t f) -> p t f", t=4 * w),
                    cos_d.rearrange("(t p) f -> p t f", p=P)[:, ts_, :])
                nc.sync.dma_start(
                    sn_all[:, fs_].rearrange("p (t f) -> p t f", t=4 * w),
                    sin_d.rearrange("(t p) f -> p t f", p=P)[:, ts_, :])

            xT = xT_pool.tile([P, FC, S], BF16, tag="xT", name="xT")
            nc.sync.dma_start(xT[:],
                              xT_d.rearrange("(f p) s -> p f s", p=P))
            wqT = wT_pool.tile([P, FC, F], BF16, tag="wqT", name="wqT")
            wkT = wT_pool.tile([P, FC, F], BF16, tag="wkT", name="wkT")
            wvT = wT_pool.tile([P, FC, F], BF16, tag="wvT", name="wvT")
            nc.scalar.dma_start(wqT[:], wqT_d.rearrange("(f p) c -> p f c", p=P))
            _trig_chunk(0, 1)
            nc.scalar.dma_start(wkT[:], wkT_d.rearrange("(f p) c -> p f c", p=P))
            nc.scalar.dma_start(wvT[:], wvT_d.rearrange("(f p) c -> p f c", p=P))
            _trig_chunk(1, 3)

            def emit_proj(s):
                sl = slice(s * P, (s + 1) * P)
                cs = cs_all[:, s * F:(s + 1) * F]
                sn = sn_all[:, s * F:(s + 1) * F]
                pq = ps_qk.tile([P, F], F32, tag="pq", name="pq")
                pk = ps_qk.tile([P, F], F32, tag="pk", name="pk")
                pv_ = ps_qk.tile([P, F], F32, tag="pv", name="pv")
                for f in range(FC):
                    st, sp = (f == 0), (f == FC - 1)
                    lhs = xT[:, f, sl]
                    nc.tensor.matmul(pq[:], lhs, wqT[:, f, :], start=st, stop=sp)
                    nc.tensor.matmul(pk[:], lhs, wkT[:, f, :], start=st, stop=sp)
                    nc.tensor.matmul(pv_[:], lhs, wvT[:, f, :], start=st, stop=sp)
                return s, sl, cs, sn, pq, pk, pv_, None

            def emit_tail(state):
                s, sl, cs, sn, pq, pk, pv_, qTs = state
                qcol = (s % 4) * P
                # RoPE (natural layout): out = q*cos + rot_half(q)*sin_signed
                for pp in (pq, pk):
                    t1 = rope_pool.tile([P, F], F32, tag="t1", name="t1")
                    nc.vector.tensor_mul(t1[:], pp[:], cs[:])
                    t2 = rope_pool.tile([P, F], F32, tag="t2", name="t2")
                    nc.vector.tensor_mul(_h3(t2)[:, :, 0:32],
                                         _h3(pp)[:, :, 32:64],
                                         _h3(sn)[:, :, 0:32])
                    nc.vector.tensor_mul(_h3(t2)[:, :, 32:64],
                                         _h3(pp)[:, :, 0:32],
                                         _h3(sn)[:, :, 32:64])
                    qr = rope_pool.tile([P, F], BF16, tag="qr", name="qr")
                    nc.gpsimd.tensor_add(qr[:], t1[:], t2[:])
                    pt = ring.tile([P, MC, P], BF16, tag="ring", name="pt")
                    for m in range(MC):
                        nc.tensor.transpose(pt[:, m, :],
                                            qr[:, m * P:(m + 1) * P],
                                            eye_sb[:])
                    if pp is pk:
                        nc.vector.tensor_copy(kT[:, :, sl], pt[:])
                    else:
                        nc.vector.tensor_copy(qTs[:, :, qcol:qcol + P], pt[:])
                # V with ones column per head: [V_h | 1] -> [P, NH*65]
                v3 = v_sb[s].rearrange("p (h e) -> p h e", h=NH)
                nc.vector.tensor_copy(v3[:, :, 0:64], _h3(pv_[:]))

            # ---- helpers for interleaved emission ----
            def attn_pair(qc, m, qTs, aoT, sums, inv6):
                # heads 2m (rows 0:64) and 2m+1 (rows 64:128): their K=64
                # scores matmuls use disjoint PE row groups -> issue adjacent
                # so the PE reorder window runs them concurrently.
                q0 = qc * QW
                last = 4 * qc + 3
                pvps = [ps_pv.tile([65, QW], F32, tag="pvp", name="pvp")
                        for _ in range(2)]
                exs = []
                for kc in range(last + 1):
                    k0 = kc * P
                    qlo = max(q0, k0)
                    n = q0 + QW - qlo
                    for par in range(2):
                        off = 64 * par
                        sp = ring.tile([P, QW], F32, tag="ring", name="sp")
                        nc.tensor.matmul(sp[:, 0:n],
                                         kT[off:off + 64, m, k0:k0 + P],
                                         qTs[off:off + 64, m,
                                             qlo - q0:qlo - q0 + n],
                                         start=True, stop=True)
                        ex = ex_pool.tile([P, QW], BF16, tag="ex", name="ex")
                        nc.scalar.activation(ex[:, 0:n], sp[:, 0:n],
                                             AF.Exp, scale=0.125,
                                             bias=ebias_sb[:])
                        if k0 >= q0:  # diagonal block: zero k > q
                            nc.gpsimd.tensor_mul(ex[:, 0:P], ex[:, 0:P],
                                                 tri_sb[:, par, :])
                        exs.append((par, kc, qlo, n, ex))
                for par, kc, qlo, n, ex in exs:
                    h = 2 * m + par
                    nc.tensor.matmul(pvps[par][:, qlo - q0:QW],
                                     v_sb[kc][:, h * 65:h * 65 + 65],
                                     ex[:, 0:n],
                                     start=(kc == 0), stop=(kc == last))
                for par in range(2):
                    h = 2 * m + par
                    off = 64 * par
                    nc.vector.tensor_copy(aoT[m][off:off + 64, :],
                                          pvps[par][0:64, :])
                    with nc.allow_low_precision(reason="softmax sums"):
                        nc.vector.reciprocal(sums[0:1, h * QW:(h + 1) * QW],
                                             pvps[par][64:65, :])
                    nc.sync.dma_start(inv6[h:h + 1, :],
                                        sums[0:1, h * QW:(h + 1) * QW])

            def attn_post(qc, aoT, sums, inv6):
                q0 = qc * QW
                for m in range(MC):
                    bp = ring.tile([P, QW], F32, tag="ring", name="bp")
                    nc.tensor.matmul(bp[:], _r(e_sb[:, m * P:(m + 1) * P]),
                                     _r(inv6[:, :]), start=True, stop=True)
                    nc.vector.tensor_mul(aoT[m][:, :], aoT[m][:, :], bp[:])
                ob = ob_pool.tile([P, QW // P, HID], BF16, tag="ob", name="ob")
                for t in range(QW // P):
                    s0 = q0 + t * P
                    for half in range(2):
                        c0, c1 = half * F, half * F + F
                        fin = ring.tile([P, F], F32, tag="ring", name="fin")
                        for m in range(MC):
                            nc.tensor.matmul(fin[:],
                                             _r(aoT[m][:, s0 - q0:s0 - q0 + P]),
                                             _r(woT[:, m, c0:c1]),
                                             start=(m == 0), stop=(m == MC - 1))
                        nc.vector.tensor_copy(ob[:, t, c0:c1], fin[:])
                nc.sync.dma_start(
                    out_d[q0:q0 + QW, :].rearrange("(t p) c -> p t c", p=P),
                    ob[:])

            def alloc_strip(qc):
                qTs = rope_pool.tile([P, MC, QW], BF16, tag="qTs", name="qTs")
                aoT = [ao_pool.tile([P, QW], BF16, tag=f"aoTs{m}", name=f"aoTs{m}")
                       for m in range(MC)]
                sums = sums_pool.tile([1, NH * QW], F32R, tag="sums", name="sums")
                inv6 = sums_pool.tile([NH, QW], F32R, tag="inv6", name="inv6")
                return qTs, aoT, sums, inv6

            # ---- emission: group 0, then zip(attention qc, proj group qc+1) ----
            strips = {0: alloc_strip(0)}
            pending = None
            pending_post = None
            for s in range(4):
                state = emit_proj(s)[:-1] + (strips[0][0],)
                if pending is not None:
                    emit_tail(pending)
                pending = state

            for qc in range(NQ):
                qTs, aoT, sums, inv6 = strips[qc]
                if qc + 1 < NQ:
                    strips[qc + 1] = alloc_strip(qc + 1)
                    nxt = list(range(4 * qc + 4, 4 * qc + 8))
                else:
                    nxt = []
                if not nxt and pending is not None:
                    emit_tail(pending)
                    pending = None
                for mp in range(MC + 1):
                    for s_i in nxt[mp:mp + 1]:
                        state = emit_proj(s_i)[:-1] + (strips[qc + 1][0],)
                        if pending is not None:
                            emit_tail(pending)
                        pending = state
                    if mp == 1 and pending_post is not None:
                        attn_post(*pending_post)
                        pending_post = None
                    if mp < MC:
                        attn_pair(qc, mp, qTs, aoT, sums, inv6)
                pending_post = (qc, aoT, sums, inv6)
            attn_post(*pending_post)
    nc.compile()
    return nc


def _rope_tables():
    inv_freq = 1.0 / (ROPE_THETA ** (np.arange(0, D, 2, dtype=np.float32) / D))
    t = np.arange(S, dtype=np.float32)
    freqs = np.outer(t, inv_freq)                       # [S, 32]
    emb = np.concatenate([freqs, freqs], axis=-1)       # [S, 64]
    cos = np.cos(emb)
    sin = np.sin(emb)
    sin_signed = sin.copy()
    sin_signed[:, 0:32] *= -1.0                         # fold rotate_half sign
    import ml_dtypes
    cos6 = np.tile(cos, (1, NH)).astype(ml_dtypes.bfloat16)
    sin6 = np.tile(sin_signed, (1, NH)).astype(ml_dtypes.bfloat16)
    return np.ascontiguousarray(cos6), np.ascontiguousarray(sin6)


_STATE = {}


def _get_program():
    if "nc" not in _STATE:
        _STATE["nc"] = build_program()
    return _STATE["nc"]


def _make_in_maps(hidden_states, Wq, Wk, Wv, Wo):
    import ml_dtypes
    bf = ml_dtypes.bfloat16

    hs = np.asarray(hidden_states, dtype=np.float32)
    Wq = np.asarray(Wq, dtype=np.float32)
    Wk = np.asarray(Wk, dtype=np.float32)
    Wv = np.asarray(Wv, dtype=np.float32)
    Wo = np.asarray(Wo, dtype=np.float32)

    cos6, sin6 = _rope_tables()
    tri = np.broadcast_to(np.triu(np.ones((P, P), dtype=bf)),
                          (2, P, P)).transpose(1, 0, 2)  # j >= i keep
    cblob = np.concatenate([
        tri.reshape(P, 2 * P),
        np.eye(P, dtype=bf),
        np.ones((P, NH), dtype=bf),
    ], axis=1).astype(bf)
    emat = np.repeat(np.eye(NH, dtype=np.float32), D, axis=1)  # [6, 384]

    in_maps = []
    for c in range(N_CORES):
        b, g = c // 2, c % 2
        cols = slice(g * F, (g + 1) * F)
        in_maps.append({
            "xT": np.ascontiguousarray(hs[b].T).astype(bf),       # [768, S]
            "wqT": np.ascontiguousarray(Wq[cols, :].T).astype(bf),  # [768, 384]
            "wkT": np.ascontiguousarray(Wk[cols, :].T).astype(bf),
            "wvT": np.ascontiguousarray(Wv[cols, :].T).astype(bf),
            "woT": np.ascontiguousarray(Wo[:, cols].T).astype(bf),  # [384, 768]
            "cos6": cos6,
            "sin6": sin6,
            "cblob": cblob,
            "emat": emat,
        })
    return in_maps


def run(hidden_states, Wq, Wk, Wv, Wo, trace=False, **trace_kw):
    nc = _get_program()
    in_maps = _make_in_maps(hidden_states, Wq, Wk, Wv, Wo)
    res = run_bass_kernel_spmd(nc, in_maps, core_ids=list(range(N_CORES)),
                               trace=trace, **trace_kw)
    B = 4
    out = np.empty((B, S, HID), dtype=np.float32)
    for b in range(B):
        out[b] = (res.results[2 * b]["out"].astype(np.float32)
                  + res.results[2 * b + 1]["out"].astype(np.float32))
    return out, res


def kernel(hidden_states, Wq, Wk, Wv, Wo):
    out, _ = run(hidden_states, Wq, Wk, Wv, Wo,
                 trace=bool(int(os.environ.get("KERNEL_TRACE", "0"))))
    return out



# revision 41
# speedup vs baseline: 1.0547x; 1.0547x over previous
"""Trainium2 Bass kernel for HNet attention (B=4, S=2048, H=768, 12 heads, RoPE, causal).

Sharding: 8 cores = 4 batches x 2 head-groups (6 heads each).
Wq/Wk/Wv split column-wise (head axis), Wo row-wise; host sums the two
partial o_proj outputs per batch (the "all-reduce" done at gather time).

Per-core dataflow (all matmuls in float32r = 1 cyc/row on the PE):
  xT [768,2048] (host-transposed) --PE--> Q,K,V natural [2048,384]
  RoPE on Q,K in natural layout (DVE/GPSIMD), PE-transpose -> QT,KT [384,2048]
  scoresT[k,q] = KT_tile.T @ QT  (per head, causal-trimmed strips)
  exp on ScalarE (no max subtraction; scores ~ N(0,1)), diag tiles masked
  PV: lhsT = [V_h | ones] [k,65] -> attn_outT rows 0:64 + softmax sums row 64
  sums -> (SBUF->SBUF DMA gather) -> reciprocal -> K=6 broadcast matmul -> scale
  o_proj: lhsT = attn_outT tiles, rhs = WoT -> out [2048,768] partial
"""

import os
import sys

import numpy as np

sys.path.insert(0, "/opt/trn_rl_repo")

from contextlib import ExitStack

import concourse.bacc as bacc
import concourse.tile as tile
from concourse import mybir
from concourse.bass_utils import run_bass_kernel_spmd

S = 2048
HID = 768
NH = 6            # heads per core
D = 64
F = NH * D        # 384 per-core feature slice
P = 128
SC = S // P       # 16
FC = HID // P     # 6
MC = F // P       # 3
QW = 512          # q strip width
NQ = S // QW      # 4
N_CORES = 8
ROPE_THETA = 10000.0

F32 = mybir.dt.float32
F32R = mybir.dt.float32r
FP8 = mybir.dt.float8e4
BF16 = mybir.dt.bfloat16
DR = mybir.MatmulPerfMode.DoubleRow
AF = mybir.ActivationFunctionType
EXP_BIAS = -2.0  # exp(s/8 - 2): keeps ex in fp8 range; cancels in softmax


def _r(ap):
    """tiles are allocated as float32r already; keep as passthrough."""
    return ap


def _h3(ap):
    """[P, F] -> [P, NH, D] view."""
    return ap.rearrange("p (h d) -> p h d", h=NH)


def build_program():
    nc = bacc.Bacc("TRN2", target_bir_lowering=False, debug=False,
                   num_devices=N_CORES)

    xT_d = nc.dram_tensor("xT", [HID, S], BF16, kind="ExternalInput").ap()
    wqT_d = nc.dram_tensor("wqT", [HID, F], BF16, kind="ExternalInput").ap()
    wkT_d = nc.dram_tensor("wkT", [HID, F], BF16, kind="ExternalInput").ap()
    wvT_d = nc.dram_tensor("wvT", [HID, F], BF16, kind="ExternalInput").ap()
    woT_d = nc.dram_tensor("woT", [F, HID], BF16, kind="ExternalInput").ap()
    cos_d = nc.dram_tensor("cos6", [S, F], BF16, kind="ExternalInput").ap()
    sin_d = nc.dram_tensor("sin6", [S, F], BF16, kind="ExternalInput").ap()
    cb_d = nc.dram_tensor("cblob", [P, 3 * P + NH], BF16,
                          kind="ExternalInput").ap()
    e_d = nc.dram_tensor("emat", [NH, F], F32R, kind="ExternalInput").ap()
    out_d = nc.dram_tensor("out", [S, HID], BF16, kind="ExternalOutput").ap()

    with tile.TileContext(nc) as tc, ExitStack() as ctx:
        const_pool = ctx.enter_context(tc.tile_pool(name="const", bufs=1))
        cb_sb = const_pool.tile([P, 3 * P + NH], BF16, tag="cblob")
        nc.sync.dma_start(cb_sb[:], cb_d[:])
        tri_sb = cb_sb[:, 0:2 * P].rearrange("p (o k) -> p o k", o=2)
        eye_sb = cb_sb[:, 2 * P:3 * P]
        on_sb = cb_sb[:, 3 * P:3 * P + NH]
        e_sb = const_pool.tile([NH, F], F32R, tag="emat")
        nc.sync.dma_start(e_sb[:], e_d[:])
        ebias_sb = const_pool.tile([P, 1], F32, tag="ebias")
        nc.gpsimd.memset(ebias_sb[:], EXP_BIAS)

        # persistent per-phase tensors
        qkT_pool = ctx.enter_context(tc.tile_pool(name="qkT", bufs=1))
        kT = qkT_pool.tile([P, MC, S], BF16, tag="kT", name="kT")
        v_pool = ctx.enter_context(tc.tile_pool(name="vp", bufs=1))
        v_sb = [v_pool.tile([P, NH * 65], BF16, tag=f"v{s}", name=f"v{s}") for s in range(SC)]
        for s in range(SC):
            nc.vector.tensor_copy(
                v_sb[s].rearrange("p (h e) -> p h e", h=NH)[:, :, 64:65],
                on_sb.rearrange("p (h o) -> p h o", h=NH))
        ao_pool = ctx.enter_context(tc.tile_pool(name="ao", bufs=2))
        woT_pool = ctx.enter_context(tc.tile_pool(name="woT", bufs=1))
        woT = woT_pool.tile([P, MC, HID], BF16, tag="woT", name="woT")
        nc.sync.dma_start(woT[:], woT_d.rearrange("(m p) c -> p m c", p=P))

        # ---- single interleaved phase: proj-group(qc) then attention(qc) ----
        # PSUM banks (8): pq1 + pk1 + ring3 (pv/pt/sc/bp) + pvp1 + fin2
        with tc.tile_pool(name="xT", bufs=1) as xT_pool, \
             tc.tile_pool(name="wT", bufs=1) as wT_pool, \
             tc.tile_pool(name="ld", bufs=2) as ld_pool, \
             tc.tile_pool(name="rope", bufs=2) as rope_pool, \
             tc.tile_pool(name="ex", bufs=9) as ex_pool, \
             tc.tile_pool(name="stg", bufs=2) as stg_pool, \
             tc.tile_pool(name="sums", bufs=2) as sums_pool, \
             tc.tile_pool(name="ob", bufs=2) as ob_pool, \
             tc.tile_pool(name="ps_qk", bufs=1, space="PSUM") as ps_qk, \
             tc.tile_pool(name="ring", bufs=3, space="PSUM") as ring, \
             tc.tile_pool(name="ps_pv", bufs=2, space="PSUM") as ps_pv:

            cs_all = xT_pool.tile([P, SC * F], BF16, tag="cosall", name="cs_all")
            sn_all = xT_pool.tile([P, SC * F], BF16, tag="sinall", name="sn_all")

            def _trig_chunk(c, w):
                ts_ = slice(c * 4, (c + w) * 4)
                fs_ = slice(c * 4 * F, (c + w) * 4 * F)
                nc.sync.dma_start(
                    cs_all[:, fs_].rearrange("p (t f) -> p t f", t=4 * w),
                    cos_d.rearrange("(t p) f -> p t f", p=P)[:, ts_, :])
                nc.sync.dma_start(
                    sn_all[:, fs_].rearrange("p (t f) -> p t f", t=4 * w),
                    sin_d.rearrange("(t p) f -> p t f", p=P)[:, ts_, :])

            xT = xT_pool.tile([P, FC, S], BF16, tag="xT", name="xT")
            nc.sync.dma_start(xT[:],
                              xT_d.rearrange("(f p) s -> p f s", p=P))
            wqT = wT_pool.tile([P, FC, F], BF16, tag="wqT", name="wqT")
            wkT = wT_pool.tile([P, FC, F], BF16, tag="wkT", name="wkT")
            wvT = wT_pool.tile([P, FC, F], BF16, tag="wvT", name="wvT")
            nc.scalar.dma_start(wqT[:], wqT_d.rearrange("(f p) c -> p f c", p=P))
            _trig_chunk(0, 1)
            nc.scalar.dma_start(wkT[:], wkT_d.rearrange("(f p) c -> p f c", p=P))
            nc.scalar.dma_start(wvT[:], wvT_d.rearrange("(f p) c -> p f c", p=P))
            _trig_chunk(1, 3)

            def emit_proj(s):
                sl = slice(s * P, (s + 1) * P)
                cs = cs_all[:, s * F:(s + 1) * F]
                sn = sn_all[:, s * F:(s + 1) * F]
                pq = ps_qk.tile([P, F], F32, tag="pq", name="pq")
                pk = ps_qk.tile([P, F], F32, tag="pk", name="pk")
                pv_ = ps_qk.tile([P, F], F32, tag="pv", name="pv")
                for f in range(FC):
                    st, sp = (f == 0), (f == FC - 1)
                    lhs = xT[:, f, sl]
                    nc.tensor.matmul(pq[:], lhs, wqT[:, f, :], start=st, stop=sp)
                    nc.tensor.matmul(pk[:], lhs, wkT[:, f, :], start=st, stop=sp)
                    nc.tensor.matmul(pv_[:], lhs, wvT[:, f, :], start=st, stop=sp)
                return s, sl, cs, sn, pq, pk, pv_, None

            def emit_tail(state):
                s, sl, cs, sn, pq, pk, pv_, qTs = state
                qcol = (s % 4) * P
                # RoPE (natural layout): out = q*cos + rot_half(q)*sin_signed
                for pp in (pq, pk):
                    t1 = rope_pool.tile([P, F], F32, tag="t1", name="t1")
                    nc.vector.tensor_mul(t1[:], pp[:], cs[:])
                    t2 = rope_pool.tile([P, F], F32, tag="t2", name="t2")
                    nc.vector.tensor_mul(_h3(t2)[:, :, 0:32],
                                         _h3(pp)[:, :, 32:64],
                                         _h3(sn)[:, :, 0:32])
                    nc.vector.tensor_mul(_h3(t2)[:, :, 32:64],
                                         _h3(pp)[:, :, 0:32],
                                         _h3(sn)[:, :, 32:64])
                    qr = rope_pool.tile([P, F], BF16, tag="qr", name="qr")
                    nc.gpsimd.tensor_add(qr[:], t1[:], t2[:])
                    pt = ring.tile([P, MC, P], BF16, tag="ring", name="pt")
                    for m in range(MC):
                        nc.tensor.transpose(pt[:, m, :],
                                            qr[:, m * P:(m + 1) * P],
                                            eye_sb[:])
                    if pp is pk:
                        nc.vector.tensor_copy(kT[:, :, sl], pt[:])
                    else:
                        nc.vector.tensor_copy(qTs[:, :, qcol:qcol + P], pt[:])
                # V with ones column per head: [V_h | 1] -> [P, NH*65]
                v3 = v_sb[s].rearrange("p (h e) -> p h e", h=NH)
                nc.vector.tensor_copy(v3[:, :, 0:64], _h3(pv_[:]))

            # ---- helpers for interleaved emission ----
            def attn_pair(qc, m, qTs, aoT, sums, inv6):
                # heads 2m (rows 0:64) and 2m+1 (rows 64:128): their K=64
                # scores matmuls use disjoint PE row groups -> issue adjacent
                # so the PE reorder window runs them concurrently.
                q0 = qc * QW
                last = 4 * qc + 3
                pvps = [ps_pv.tile([65, QW], F32, tag="pvp", name="pvp")
                        for _ in range(2)]
                exs = []
                for kc in range(last + 1):
                    k0 = kc * P
                    qlo = max(q0, k0)
                    n = q0 + QW - qlo
                    for par in range(2):
                        off = 64 * par
                        sp = ring.tile([P, QW], F32, tag="ring", name="sp")
                        nc.tensor.matmul(sp[:, 0:n],
                                         kT[off:off + 64, m, k0:k0 + P],
                                         qTs[off:off + 64, m,
                                             qlo - q0:qlo - q0 + n],
                                         start=True, stop=True)
                        ex = ex_pool.tile([P, QW], BF16, tag="ex", name="ex")
                        nc.scalar.activation(ex[:, 0:n], sp[:, 0:n],
                                             AF.Exp, scale=0.125,
                                             bias=ebias_sb[:])
                        if k0 >= q0:  # diagonal block: zero k > q
                            nc.gpsimd.tensor_mul(ex[:, 0:P], ex[:, 0:P],
                                                 tri_sb[:, par, :])
                        exs.append((par, kc, qlo, n, ex))
                for par, kc, qlo, n, ex in exs:
                    h = 2 * m + par
                    nc.tensor.matmul(pvps[par][:, qlo - q0:QW],
                                     v_sb[kc][:, h * 65:h * 65 + 65],
                                     ex[:, 0:n],
                                     start=(kc == 0), stop=(kc == last))
                for par in range(2):
                    h = 2 * m + par
                    off = 64 * par
                    if qc < 2:
                        nc.scalar.copy(aoT[m][off:off + 64, :],
                                       pvps[par][0:64, :])
                    else:
                        nc.vector.tensor_copy(aoT[m][off:off + 64, :],
                                              pvps[par][0:64, :])
                    with nc.allow_low_precision(reason="softmax sums"):
                        nc.vector.reciprocal(sums[0:1, h * QW:(h + 1) * QW],
                                             pvps[par][64:65, :])
                    nc.sync.dma_start(inv6[h:h + 1, :],
                                        sums[0:1, h * QW:(h + 1) * QW])

            def attn_post(qc, aoT, sums, inv6):
                q0 = qc * QW
                for m in range(MC):
                    bp = ring.tile([P, QW], F32, tag="ring", name="bp")
                    nc.tensor.matmul(bp[:], _r(e_sb[:, m * P:(m + 1) * P]),
                                     _r(inv6[:, :]), start=True, stop=True)
                    nc.vector.tensor_mul(aoT[m][:, :], aoT[m][:, :], bp[:])
                ob = ob_pool.tile([P, QW // P, HID], BF16, tag="ob", name="ob")
                for t in range(QW // P):
                    s0 = q0 + t * P
                    for half in range(2):
                        c0, c1 = half * F, half * F + F
                        fin = ring.tile([P, F], F32, tag="ring", name="fin")
                        for m in range(MC):
                            nc.tensor.matmul(fin[:],
                                             _r(aoT[m][:, s0 - q0:s0 - q0 + P]),
                                             _r(woT[:, m, c0:c1]),
                                             start=(m == 0), stop=(m == MC - 1))
                        nc.vector.tensor_copy(ob[:, t, c0:c1], fin[:])
                half = QW // (2 * P)
                for g in range(2):
                    r0 = q0 + g * (QW // 2)
                    nc.sync.dma_start(
                        out_d[r0:r0 + QW // 2, :].rearrange(
                            "(t p) c -> p t c", p=P),
                        ob[:, g * half:(g + 1) * half, :])

            def alloc_strip(qc):
                qTs = rope_pool.tile([P, MC, QW], BF16, tag="qTs", name="qTs")
                aoT = [ao_pool.tile([P, QW], BF16, tag=f"aoTs{m}", name=f"aoTs{m}")
                       for m in range(MC)]
                sums = sums_pool.tile([1, NH * QW], F32R, tag="sums", name="sums")
                inv6 = sums_pool.tile([NH, QW], F32R, tag="inv6", name="inv6")
                return qTs, aoT, sums, inv6

            # ---- emission: group 0, then zip(attention qc, proj group qc+1) ----
            strips = {0: alloc_strip(0)}
            pending = None
            pending_post = None
            for s in range(4):
                state = emit_proj(s)[:-1] + (strips[0][0],)
                if pending is not None:
                    emit_tail(pending)
                pending = state

            for qc in range(NQ):
                qTs, aoT, sums, inv6 = strips[qc]
                if qc + 1 < NQ:
                    strips[qc + 1] = alloc_strip(qc + 1)
                    nxt = list(range(4 * qc + 4, 4 * qc + 8))
                else:
                    nxt = []
                if not nxt and pending is not None:
                    emit_tail(pending)
                    pending = None
                for mp in range(MC + 1):
                    for s_i in nxt[mp:mp + 1]:
                        state = emit_proj(s_i)[:-1] + (strips[qc + 1][0],)
                        if pending is not None:
                            emit_tail(pending)
                        pending = state
                    if mp == 1 and pending_post is not None:
                        attn_post(*pending_post)
                        pending_post = None
                    if mp < MC:
                        attn_pair(qc, mp, qTs, aoT, sums, inv6)
                pending_post = (qc, aoT, sums, inv6)
            attn_post(*pending_post)
    nc.compile()
    return nc


def _rope_tables():
    inv_freq = 1.0 / (ROPE_THETA ** (np.arange(0, D, 2, dtype=np.float32) / D))
    t = np.arange(S, dtype=np.float32)
    freqs = np.outer(t, inv_freq)                       # [S, 32]
    emb = np.concatenate([freqs, freqs], axis=-1)       # [S, 64]
    cos = np.cos(emb)
    sin = np.sin(emb)
    sin_signed = sin.copy()
    sin_signed[:, 0:32] *= -1.0                         # fold rotate_half sign
    import ml_dtypes
    cos6 = np.tile(cos, (1, NH)).astype(ml_dtypes.bfloat16)
    sin6 = np.tile(sin_signed, (1, NH)).astype(ml_dtypes.bfloat16)
    return np.ascontiguousarray(cos6), np.ascontiguousarray(sin6)


_STATE = {}


def _get_program():
    if "nc" not in _STATE:
        _STATE["nc"] = build_program()
    return _STATE["nc"]


def _make_in_maps(hidden_states, Wq, Wk, Wv, Wo):
    import ml_dtypes
    bf = ml_dtypes.bfloat16

    hs = np.asarray(hidden_states, dtype=np.float32)
    Wq = np.asarray(Wq, dtype=np.float32)
    Wk = np.asarray(Wk, dtype=np.float32)
    Wv = np.asarray(Wv, dtype=np.float32)
    Wo = np.asarray(Wo, dtype=np.float32)

    cos6, sin6 = _rope_tables()
    tri = np.broadcast_to(np.triu(np.ones((P, P), dtype=bf)),
                          (2, P, P)).transpose(1, 0, 2)  # j >= i keep
    cblob = np.concatenate([
        tri.reshape(P, 2 * P),
        np.eye(P, dtype=bf),
        np.ones((P, NH), dtype=bf),
    ], axis=1).astype(bf)
    emat = np.repeat(np.eye(NH, dtype=np.float32), D, axis=1)  # [6, 384]

    in_maps = []
    for c in range(N_CORES):
        b, g = c // 2, c % 2
        cols = slice(g * F, (g + 1) * F)
        in_maps.append({
            "xT": np.ascontiguousarray(hs[b].T).astype(bf),       # [768, S]
            "wqT": np.ascontiguousarray(Wq[cols, :].T).astype(bf),  # [768, 384]
            "wkT": np.ascontiguousarray(Wk[cols, :].T).astype(bf),
            "wvT": np.ascontiguousarray(Wv[cols, :].T).astype(bf),
            "woT": np.ascontiguousarray(Wo[:, cols].T).astype(bf),  # [384, 768]
            "cos6": cos6,
            "sin6": sin6,
            "cblob": cblob,
            "emat": emat,
        })
    return in_maps


def run(hidden_states, Wq, Wk, Wv, Wo, trace=False, **trace_kw):
    nc = _get_program()
    in_maps = _make_in_maps(hidden_states, Wq, Wk, Wv, Wo)
    res = run_bass_kernel_spmd(nc, in_maps, core_ids=list(range(N_CORES)),
                               trace=trace, **trace_kw)
    B = 4
    out = np.empty((B, S, HID), dtype=np.float32)
    for b in range(B):
        out[b] = (res.results[2 * b]["out"].astype(np.float32)
                  + res.results[2 * b + 1]["out"].astype(np.float32))
    return out, res


def kernel(hidden_states, Wq, Wk, Wv, Wo):
    out, _ = run(hidden_states, Wq, Wk, Wv, Wo,
                 trace=bool(int(os.environ.get("KERNEL_TRACE", "0"))))
    return out



# revision 44
# speedup vs baseline: 1.0573x; 1.0024x over previous
"""Trainium2 Bass kernel for HNet attention (B=4, S=2048, H=768, 12 heads, RoPE, causal).

Sharding: 8 cores = 4 batches x 2 head-groups (6 heads each).
Wq/Wk/Wv split column-wise (head axis), Wo row-wise; host sums the two
partial o_proj outputs per batch (the "all-reduce" done at gather time).

Per-core dataflow (all matmuls in float32r = 1 cyc/row on the PE):
  xT [768,2048] (host-transposed) --PE--> Q,K,V natural [2048,384]
  RoPE on Q,K in natural layout (DVE/GPSIMD), PE-transpose -> QT,KT [384,2048]
  scoresT[k,q] = KT_tile.T @ QT  (per head, causal-trimmed strips)
  exp on ScalarE (no max subtraction; scores ~ N(0,1)), diag tiles masked
  PV: lhsT = [V_h | ones] [k,65] -> attn_outT rows 0:64 + softmax sums row 64
  sums -> (SBUF->SBUF DMA gather) -> reciprocal -> K=6 broadcast matmul -> scale
  o_proj: lhsT = attn_outT tiles, rhs = WoT -> out [2048,768] partial
"""

import os
import sys

import numpy as np

sys.path.insert(0, "/opt/trn_rl_repo")

from contextlib import ExitStack

import concourse.bacc as bacc
import concourse.tile as tile
from concourse import mybir
from concourse.bass_utils import run_bass_kernel_spmd

S = 2048
HID = 768
NH = 6            # heads per core
D = 64
F = NH * D        # 384 per-core feature slice
P = 128
SC = S // P       # 16
FC = HID // P     # 6
MC = F // P       # 3
QW = 512          # q strip width
NQ = S // QW      # 4
N_CORES = 8
ROPE_THETA = 10000.0

F32 = mybir.dt.float32
F32R = mybir.dt.float32r
FP8 = mybir.dt.float8e4
BF16 = mybir.dt.bfloat16
DR = mybir.MatmulPerfMode.DoubleRow
AF = mybir.ActivationFunctionType
EXP_BIAS = -2.0  # exp(s/8 - 2): keeps ex in fp8 range; cancels in softmax


def _r(ap):
    """tiles are allocated as float32r already; keep as passthrough."""
    return ap


def _h3(ap):
    """[P, F] -> [P, NH, D] view."""
    return ap.rearrange("p (h d) -> p h d", h=NH)


def build_program():
    nc = bacc.Bacc("TRN2", target_bir_lowering=False, debug=False,
                   num_devices=N_CORES)

    xT_d = nc.dram_tensor("xT", [HID, S], BF16, kind="ExternalInput").ap()
    wqT_d = nc.dram_tensor("wqT", [HID, F], BF16, kind="ExternalInput").ap()
    wkT_d = nc.dram_tensor("wkT", [HID, F], BF16, kind="ExternalInput").ap()
    wvT_d = nc.dram_tensor("wvT", [HID, F], BF16, kind="ExternalInput").ap()
    woT_d = nc.dram_tensor("woT", [F, HID], BF16, kind="ExternalInput").ap()
    cos_d = nc.dram_tensor("cos6", [S, F], BF16, kind="ExternalInput").ap()
    sin_d = nc.dram_tensor("sin6", [S, F], BF16, kind="ExternalInput").ap()
    cb_d = nc.dram_tensor("cblob", [P, 3 * P + NH], BF16,
                          kind="ExternalInput").ap()
    e_d = nc.dram_tensor("emat", [NH, F], F32R, kind="ExternalInput").ap()
    out_d = nc.dram_tensor("out", [S, HID], BF16, kind="ExternalOutput").ap()

    with tile.TileContext(nc) as tc, ExitStack() as ctx:
        const_pool = ctx.enter_context(tc.tile_pool(name="const", bufs=1))
        cb_sb = const_pool.tile([P, 3 * P + NH], BF16, tag="cblob")
        nc.sync.dma_start(cb_sb[:], cb_d[:])
        tri_sb = cb_sb[:, 0:2 * P].rearrange("p (o k) -> p o k", o=2)
        eye_sb = cb_sb[:, 2 * P:3 * P]
        on_sb = cb_sb[:, 3 * P:3 * P + NH]
        e_sb = const_pool.tile([NH, F], F32R, tag="emat")
        nc.sync.dma_start(e_sb[:], e_d[:])
        ebias_sb = const_pool.tile([P, 1], F32, tag="ebias")
        nc.gpsimd.memset(ebias_sb[:], EXP_BIAS)

        # persistent per-phase tensors
        qkT_pool = ctx.enter_context(tc.tile_pool(name="qkT", bufs=1))
        kT = qkT_pool.tile([P, MC, S], BF16, tag="kT", name="kT")
        v_pool = ctx.enter_context(tc.tile_pool(name="vp", bufs=1))
        v_sb = [v_pool.tile([P, NH * 65], BF16, tag=f"v{s}", name=f"v{s}") for s in range(SC)]
        for s in range(SC):
            nc.vector.tensor_copy(
                v_sb[s].rearrange("p (h e) -> p h e", h=NH)[:, :, 64:65],
                on_sb.rearrange("p (h o) -> p h o", h=NH))
        ao_pool = ctx.enter_context(tc.tile_pool(name="ao", bufs=2))
        woT_pool = ctx.enter_context(tc.tile_pool(name="woT", bufs=1))
        woT = woT_pool.tile([P, MC, HID], BF16, tag="woT", name="woT")
        nc.sync.dma_start(woT[:], woT_d.rearrange("(m p) c -> p m c", p=P))

        # ---- single interleaved phase: proj-group(qc) then attention(qc) ----
        # PSUM banks (8): pq1 + pk1 + ring3 (pv/pt/sc/bp) + pvp1 + fin2
        with tc.tile_pool(name="xT", bufs=1) as xT_pool, \
             tc.tile_pool(name="wT", bufs=1) as wT_pool, \
             tc.tile_pool(name="ld", bufs=2) as ld_pool, \
             tc.tile_pool(name="rope", bufs=2) as rope_pool, \
             tc.tile_pool(name="ex", bufs=9) as ex_pool, \
             tc.tile_pool(name="stg", bufs=2) as stg_pool, \
             tc.tile_pool(name="sums", bufs=2) as sums_pool, \
             tc.tile_pool(name="ob", bufs=2) as ob_pool, \
             tc.tile_pool(name="ps_qk", bufs=1, space="PSUM") as ps_qk, \
             tc.tile_pool(name="ring", bufs=3, space="PSUM") as ring, \
             tc.tile_pool(name="ps_pv", bufs=2, space="PSUM") as ps_pv:

            cs_all = xT_pool.tile([P, SC * F], BF16, tag="cosall", name="cs_all")
            sn_all = xT_pool.tile([P, SC * F], BF16, tag="sinall", name="sn_all")

            def _trig_chunk(c, w):
                ts_ = slice(c * 4, (c + w) * 4)
                fs_ = slice(c * 4 * F, (c + w) * 4 * F)
                nc.sync.dma_start(
                    cs_all[:, fs_].rearrange("p (t f) -> p t f", t=4 * w),
                    cos_d.rearrange("(t p) f -> p t f", p=P)[:, ts_, :])
                nc.sync.dma_start(
                    sn_all[:, fs_].rearrange("p (t f) -> p t f", t=4 * w),
                    sin_d.rearrange("(t p) f -> p t f", p=P)[:, ts_, :])

            xT = xT_pool.tile([P, FC, S], BF16, tag="xT", name="xT")
            nc.sync.dma_start(xT[:],
                              xT_d.rearrange("(f p) s -> p f s", p=P))
            wqT = wT_pool.tile([P, FC, F], BF16, tag="wqT", name="wqT")
            wkT = wT_pool.tile([P, FC, F], BF16, tag="wkT", name="wkT")
            wvT = wT_pool.tile([P, FC, F], BF16, tag="wvT", name="wvT")
            nc.scalar.dma_start(wqT[:], wqT_d.rearrange("(f p) c -> p f c", p=P))
            _trig_chunk(0, 1)
            nc.scalar.dma_start(wkT[:], wkT_d.rearrange("(f p) c -> p f c", p=P))
            nc.scalar.dma_start(wvT[:], wvT_d.rearrange("(f p) c -> p f c", p=P))
            _trig_chunk(1, 3)

            def emit_proj(s):
                sl = slice(s * P, (s + 1) * P)
                cs = cs_all[:, s * F:(s + 1) * F]
                sn = sn_all[:, s * F:(s + 1) * F]
                pq = ps_qk.tile([P, F], F32, tag="pq", name="pq")
                pk = ps_qk.tile([P, F], F32, tag="pk", name="pk")
                pv_ = ps_qk.tile([P, F], F32, tag="pv", name="pv")
                for f in range(FC):
                    st, sp = (f == 0), (f == FC - 1)
                    lhs = xT[:, f, sl]
                    nc.tensor.matmul(pq[:], lhs, wqT[:, f, :], start=st, stop=sp)
                    nc.tensor.matmul(pk[:], lhs, wkT[:, f, :], start=st, stop=sp)
                    nc.tensor.matmul(pv_[:], lhs, wvT[:, f, :], start=st, stop=sp)
                return s, sl, cs, sn, pq, pk, pv_, None

            def emit_tail(state):
                s, sl, cs, sn, pq, pk, pv_, qTs = state
                qcol = (s % 4) * P
                # RoPE (natural layout): out = q*cos + rot_half(q)*sin_signed
                for pp in (pq, pk):
                    t1 = rope_pool.tile([P, F], F32, tag="t1", name="t1")
                    nc.vector.tensor_mul(t1[:], pp[:], cs[:])
                    t2 = rope_pool.tile([P, F], F32, tag="t2", name="t2")
                    nc.vector.tensor_mul(_h3(t2)[:, :, 0:32],
                                         _h3(pp)[:, :, 32:64],
                                         _h3(sn)[:, :, 0:32])
                    nc.vector.tensor_mul(_h3(t2)[:, :, 32:64],
                                         _h3(pp)[:, :, 0:32],
                                         _h3(sn)[:, :, 32:64])
                    qr = rope_pool.tile([P, F], BF16, tag="qr", name="qr")
                    nc.gpsimd.tensor_add(qr[:], t1[:], t2[:])
                    pt = ring.tile([P, MC, P], BF16, tag="ring", name="pt")
                    for m in range(MC):
                        nc.tensor.transpose(pt[:, m, :],
                                            qr[:, m * P:(m + 1) * P],
                                            eye_sb[:])
                    if pp is pk:
                        nc.vector.tensor_copy(kT[:, :, sl], pt[:])
                    else:
                        nc.vector.tensor_copy(qTs[:, :, qcol:qcol + P], pt[:])
                # V with ones column per head: [V_h | 1] -> [P, NH*65]
                v3 = v_sb[s].rearrange("p (h e) -> p h e", h=NH)
                nc.vector.tensor_copy(v3[:, :, 0:64], _h3(pv_[:]))

            # ---- helpers for interleaved emission ----
            def attn_pair(qc, m, qTs, aoT, sums, inv6):
                # heads 2m (rows 0:64) and 2m+1 (rows 64:128): their K=64
                # scores matmuls use disjoint PE row groups -> issue adjacent
                # so the PE reorder window runs them concurrently.
                q0 = qc * QW
                last = 4 * qc + 3
                pvps = [ps_pv.tile([65, QW], F32, tag="pvp", name="pvp")
                        for _ in range(2)]
                exs = []
                for kc in range(last + 1):
                    k0 = kc * P
                    qlo = max(q0, k0)
                    n = q0 + QW - qlo
                    for par in range(2):
                        off = 64 * par
                        sp = ring.tile([P, QW], F32, tag="ring", name="sp")
                        nc.tensor.matmul(sp[:, 0:n],
                                         kT[off:off + 64, m, k0:k0 + P],
                                         qTs[off:off + 64, m,
                                             qlo - q0:qlo - q0 + n],
                                         start=True, stop=True)
                        ex = ex_pool.tile([P, QW], BF16, tag="ex", name="ex")
                        nc.scalar.activation(ex[:, 0:n], sp[:, 0:n],
                                             AF.Exp, scale=0.125,
                                             bias=ebias_sb[:])
                        if k0 >= q0:  # diagonal block: zero k > q
                            nc.gpsimd.tensor_mul(ex[:, 0:P], ex[:, 0:P],
                                                 tri_sb[:, par, :])
                        exs.append((par, kc, qlo, n, ex))
                for par, kc, qlo, n, ex in exs:
                    h = 2 * m + par
                    nc.tensor.matmul(pvps[par][:, qlo - q0:QW],
                                     v_sb[kc][:, h * 65:h * 65 + 65],
                                     ex[:, 0:n],
                                     start=(kc == 0), stop=(kc == last))
                for par in range(2):
                    h = 2 * m + par
                    off = 64 * par
                    if qc < 2:
                        nc.scalar.copy(aoT[m][off:off + 64, :],
                                       pvps[par][0:64, :])
                    else:
                        nc.vector.tensor_copy(aoT[m][off:off + 64, :],
                                              pvps[par][0:64, :])
                    with nc.allow_low_precision(reason="softmax sums"):
                        nc.vector.reciprocal(sums[0:1, h * QW:(h + 1) * QW],
                                             pvps[par][64:65, :])
                    if m == MC - 1 and par == 1:
                        nc.sync.dma_start(inv6[:], sums[0:1, :])

            def attn_post(qc, aoT, sums, inv6):
                q0 = qc * QW
                for m in range(MC):
                    bp = ring.tile([P, QW], F32, tag="ring", name="bp")
                    nc.tensor.matmul(bp[:], _r(e_sb[:, m * P:(m + 1) * P]),
                                     _r(inv6[:, :]), start=True, stop=True)
                    nc.vector.tensor_mul(aoT[m][:, :], aoT[m][:, :], bp[:])
                ob = ob_pool.tile([P, QW // P, HID], BF16, tag="ob", name="ob")
                for t in range(QW // P):
                    s0 = q0 + t * P
                    for half in range(2):
                        c0, c1 = half * F, half * F + F
                        fin = ring.tile([P, F], F32, tag="ring", name="fin")
                        for m in range(MC):
                            nc.tensor.matmul(fin[:],
                                             _r(aoT[m][:, s0 - q0:s0 - q0 + P]),
                                             _r(woT[:, m, c0:c1]),
                                             start=(m == 0), stop=(m == MC - 1))
                        if qc < 2:
                            nc.scalar.copy(ob[:, t, c0:c1], fin[:])
                        else:
                            nc.vector.tensor_copy(ob[:, t, c0:c1], fin[:])
                half = QW // (2 * P)
                for g in range(2):
                    r0 = q0 + g * (QW // 2)
                    nc.sync.dma_start(
                        out_d[r0:r0 + QW // 2, :].rearrange(
                            "(t p) c -> p t c", p=P),
                        ob[:, g * half:(g + 1) * half, :])

            def alloc_strip(qc):
                qTs = rope_pool.tile([P, MC, QW], BF16, tag="qTs", name="qTs")
                aoT = [ao_pool.tile([P, QW], BF16, tag=f"aoTs{m}", name=f"aoTs{m}")
                       for m in range(MC)]
                sums = sums_pool.tile([1, NH * QW], F32R, tag="sums", name="sums")
                inv6 = sums_pool.tile([NH, QW], F32R, tag="inv6", name="inv6")
                return qTs, aoT, sums, inv6

            # ---- emission: group 0, then zip(attention qc, proj group qc+1) ----
            strips = {0: alloc_strip(0)}
            pending = None
            pending_post = None
            for s in range(4):
                state = emit_proj(s)[:-1] + (strips[0][0],)
                if pending is not None:
                    emit_tail(pending)
                pending = state

            for qc in range(NQ):
                qTs, aoT, sums, inv6 = strips[qc]
                if qc + 1 < NQ:
                    strips[qc + 1] = alloc_strip(qc + 1)
                    nxt = list(range(4 * qc + 4, 4 * qc + 8))
                else:
                    nxt = []
                if not nxt and pending is not None:
                    emit_tail(pending)
                    pending = None
                for mp in range(MC + 1):
                    for s_i in nxt[mp:mp + 1]:
                        state = emit_proj(s_i)[:-1] + (strips[qc + 1][0],)
                        if pending is not None:
                            emit_tail(pending)
                        pending = state
                    if mp == 1 and pending_post is not None:
                        attn_post(*pending_post)
                        pending_post = None
                    if mp < MC:
                        attn_pair(qc, mp, qTs, aoT, sums, inv6)
                pending_post = (qc, aoT, sums, inv6)
            attn_post(*pending_post)
    nc.compile()
    return nc


def _rope_tables():
    inv_freq = 1.0 / (ROPE_THETA ** (np.arange(0, D, 2, dtype=np.float32) / D))
    t = np.arange(S, dtype=np.float32)
    freqs = np.outer(t, inv_freq)                       # [S, 32]
    emb = np.concatenate([freqs, freqs], axis=-1)       # [S, 64]
    cos = np.cos(emb)
    sin = np.sin(emb)
    sin_signed = sin.copy()
    sin_signed[:, 0:32] *= -1.0                         # fold rotate_half sign
    import ml_dtypes
    cos6 = np.tile(cos, (1, NH)).astype(ml_dtypes.bfloat16)
    sin6 = np.tile(sin_signed, (1, NH)).astype(ml_dtypes.bfloat16)
    return np.ascontiguousarray(cos6), np.ascontiguousarray(sin6)


_STATE = {}


def _get_program():
    if "nc" not in _STATE:
        _STATE["nc"] = build_program()
    return _STATE["nc"]


def _make_in_maps(hidden_states, Wq, Wk, Wv, Wo):
    import ml_dtypes
    bf = ml_dtypes.bfloat16

    hs = np.asarray(hidden_states, dtype=np.float32)
    Wq = np.asarray(Wq, dtype=np.float32)
    Wk = np.asarray(Wk, dtype=np.float32)
    Wv = np.asarray(Wv, dtype=np.float32)
    Wo = np.asarray(Wo, dtype=np.float32)

    cos6, sin6 = _rope_tables()
    tri = np.broadcast_to(np.triu(np.ones((P, P), dtype=bf)),
                          (2, P, P)).transpose(1, 0, 2)  # j >= i keep
    cblob = np.concatenate([
        tri.reshape(P, 2 * P),
        np.eye(P, dtype=bf),
        np.ones((P, NH), dtype=bf),
    ], axis=1).astype(bf)
    emat = np.repeat(np.eye(NH, dtype=np.float32), D, axis=1)  # [6, 384]

    in_maps = []
    for c in range(N_CORES):
        b, g = c // 2, c % 2
        cols = slice(g * F, (g + 1) * F)
        in_maps.append({
            "xT": np.ascontiguousarray(hs[b].T).astype(bf),       # [768, S]
            "wqT": np.ascontiguousarray(Wq[cols, :].T).astype(bf),  # [768, 384]
            "wkT": np.ascontiguousarray(Wk[cols, :].T).astype(bf),
            "wvT": np.ascontiguousarray(Wv[cols, :].T).astype(bf),
            "woT": np.ascontiguousarray(Wo[:, cols].T).astype(bf),  # [384, 768]
            "cos6": cos6,
            "sin6": sin6,
            "cblob": cblob,
            "emat": emat,
        })
    return in_maps


def run(hidden_states, Wq, Wk, Wv, Wo, trace=False, **trace_kw):
    nc = _get_program()
    in_maps = _make_in_maps(hidden_states, Wq, Wk, Wv, Wo)
    res = run_bass_kernel_spmd(nc, in_maps, core_ids=list(range(N_CORES)),
                               trace=trace, **trace_kw)
    B = 4
    out = np.empty((B, S, HID), dtype=np.float32)
    for b in range(B):
        out[b] = (res.results[2 * b]["out"].astype(np.float32)
                  + res.results[2 * b + 1]["out"].astype(np.float32))
    return out, res


def kernel(hidden_states, Wq, Wk, Wv, Wo):
    out, _ = run(hidden_states, Wq, Wk, Wv, Wo,
                 trace=bool(int(os.environ.get("KERNEL_TRACE", "0"))))
    return out



# revision 45
# speedup vs baseline: 1.1140x; 1.0536x over previous
"""Trainium2 Bass kernel for HNet attention (B=4, S=2048, H=768, 12 heads, RoPE, causal).

Sharding: 8 cores = 4 batches x 2 head-groups (6 heads each).
Wq/Wk/Wv split column-wise (head axis), Wo row-wise; host sums the two
partial o_proj outputs per batch (the "all-reduce" done at gather time).

Per-core dataflow (all matmuls in float32r = 1 cyc/row on the PE):
  xT [768,2048] (host-transposed) --PE--> Q,K,V natural [2048,384]
  RoPE on Q,K in natural layout (DVE/GPSIMD), PE-transpose -> QT,KT [384,2048]
  scoresT[k,q] = KT_tile.T @ QT  (per head, causal-trimmed strips)
  exp on ScalarE (no max subtraction; scores ~ N(0,1)), diag tiles masked
  PV: lhsT = [V_h | ones] [k,65] -> attn_outT rows 0:64 + softmax sums row 64
  sums -> (SBUF->SBUF DMA gather) -> reciprocal -> K=6 broadcast matmul -> scale
  o_proj: lhsT = attn_outT tiles, rhs = WoT -> out [2048,768] partial
"""

import os
import sys

import numpy as np

sys.path.insert(0, "/opt/trn_rl_repo")

from contextlib import ExitStack

import concourse.bacc as bacc
import concourse.tile as tile
from concourse import mybir
from concourse.bass_utils import run_bass_kernel_spmd

S = 2048
HID = 768
NH = 6            # heads per core
D = 64
F = NH * D        # 384 per-core feature slice
P = 128
SC = S // P       # 16
FC = HID // P     # 6
MC = F // P       # 3
QW = 512          # q strip width
NQ = S // QW      # 4
N_CORES = 8
ROPE_THETA = 10000.0

F32 = mybir.dt.float32
F32R = mybir.dt.float32r
FP8 = mybir.dt.float8e4
BF16 = mybir.dt.bfloat16
DR = mybir.MatmulPerfMode.DoubleRow
AF = mybir.ActivationFunctionType
EXP_BIAS = -2.0  # exp(s/8 - 2): keeps ex in fp8 range; cancels in softmax


def _r(ap):
    """tiles are allocated as float32r already; keep as passthrough."""
    return ap


def _h3(ap):
    """[P, F] -> [P, NH, D] view."""
    return ap.rearrange("p (h d) -> p h d", h=NH)


def build_program():
    nc = bacc.Bacc("TRN2", target_bir_lowering=False, debug=False,
                   num_devices=N_CORES)

    xT_d = nc.dram_tensor("xT", [HID, S], BF16, kind="ExternalInput").ap()
    wqT_d = nc.dram_tensor("wqT", [HID, F], BF16, kind="ExternalInput").ap()
    wkT_d = nc.dram_tensor("wkT", [HID, F], BF16, kind="ExternalInput").ap()
    wvT_d = nc.dram_tensor("wvT", [HID, F], BF16, kind="ExternalInput").ap()
    woT_d = nc.dram_tensor("woT", [F, HID], BF16, kind="ExternalInput").ap()
    cos_d = nc.dram_tensor("cos6", [S, F], BF16, kind="ExternalInput").ap()
    sin_d = nc.dram_tensor("sin6", [S, F], BF16, kind="ExternalInput").ap()
    cb_d = nc.dram_tensor("cblob", [P, 3 * P + NH], BF16,
                          kind="ExternalInput").ap()
    e_d = nc.dram_tensor("emat", [NH, F], F32R, kind="ExternalInput").ap()
    out_d = nc.dram_tensor("out", [S, HID], BF16, kind="ExternalOutput").ap()

    with tile.TileContext(nc) as tc, ExitStack() as ctx:
        const_pool = ctx.enter_context(tc.tile_pool(name="const", bufs=1))
        cb_sb = const_pool.tile([P, 3 * P + NH], BF16, tag="cblob")
        nc.sync.dma_start(cb_sb[:], cb_d[:])
        tri_sb = cb_sb[:, 0:2 * P].rearrange("p (o k) -> p o k", o=2)
        eye_sb = cb_sb[:, 2 * P:3 * P]
        on_sb = cb_sb[:, 3 * P:3 * P + NH]
        e_sb = const_pool.tile([NH, F], F32R, tag="emat")
        nc.sync.dma_start(e_sb[:], e_d[:])
        ebias_sb = const_pool.tile([P, 1], F32, tag="ebias")
        nc.gpsimd.memset(ebias_sb[:], EXP_BIAS)

        # persistent per-phase tensors
        qkT_pool = ctx.enter_context(tc.tile_pool(name="qkT", bufs=1))
        kT = qkT_pool.tile([P, MC, S], BF16, tag="kT", name="kT")
        v_pool = ctx.enter_context(tc.tile_pool(name="vp", bufs=1))
        v_sb = [v_pool.tile([P, NH * 65], BF16, tag=f"v{s}", name=f"v{s}") for s in range(SC)]
        for s in range(SC):
            nc.vector.tensor_copy(
                v_sb[s].rearrange("p (h e) -> p h e", h=NH)[:, :, 64:65],
                on_sb.rearrange("p (h o) -> p h o", h=NH))
        ao_pool = ctx.enter_context(tc.tile_pool(name="ao", bufs=2))
        woT_pool = ctx.enter_context(tc.tile_pool(name="woT", bufs=1))
        woT = woT_pool.tile([P, MC, HID], BF16, tag="woT", name="woT")

        # ---- single interleaved phase: proj-group(qc) then attention(qc) ----
        # PSUM banks (8): pq1 + pk1 + ring3 (pv/pt/sc/bp) + pvp1 + fin2
        with tc.tile_pool(name="xT", bufs=1) as xT_pool, \
             tc.tile_pool(name="wT", bufs=1) as wT_pool, \
             tc.tile_pool(name="ld", bufs=2) as ld_pool, \
             tc.tile_pool(name="rope", bufs=2) as rope_pool, \
             tc.tile_pool(name="ex", bufs=9) as ex_pool, \
             tc.tile_pool(name="stg", bufs=2) as stg_pool, \
             tc.tile_pool(name="sums", bufs=2) as sums_pool, \
             tc.tile_pool(name="ob", bufs=2) as ob_pool, \
             tc.tile_pool(name="ps_qk", bufs=1, space="PSUM") as ps_qk, \
             tc.tile_pool(name="ring", bufs=3, space="PSUM") as ring, \
             tc.tile_pool(name="ps_pv", bufs=2, space="PSUM") as ps_pv:

            cs_all = xT_pool.tile([P, SC * F], BF16, tag="cosall", name="cs_all")
            sn_all = xT_pool.tile([P, SC * F], BF16, tag="sinall", name="sn_all")

            def _trig_chunk(c, w):
                ts_ = slice(c * 4, (c + w) * 4)
                fs_ = slice(c * 4 * F, (c + w) * 4 * F)
                nc.sync.dma_start(
                    cs_all[:, fs_].rearrange("p (t f) -> p t f", t=4 * w),
                    cos_d.rearrange("(t p) f -> p t f", p=P)[:, ts_, :])
                nc.sync.dma_start(
                    sn_all[:, fs_].rearrange("p (t f) -> p t f", t=4 * w),
                    sin_d.rearrange("(t p) f -> p t f", p=P)[:, ts_, :])

            xT = xT_pool.tile([P, FC, S], BF16, tag="xT", name="xT")
            xT_src = xT_d.rearrange("(f p) s -> p f s", p=P)
            nc.sync.dma_start(xT[:, :, 0:QW], xT_src[:, :, 0:QW])
            wqT = wT_pool.tile([P, FC, F], BF16, tag="wqT", name="wqT")
            wkT = wT_pool.tile([P, FC, F], BF16, tag="wkT", name="wkT")
            wvT = wT_pool.tile([P, FC, F], BF16, tag="wvT", name="wvT")
            nc.scalar.dma_start(wqT[:], wqT_d.rearrange("(f p) c -> p f c", p=P))
            nc.scalar.dma_start(wkT[:], wkT_d.rearrange("(f p) c -> p f c", p=P))
            nc.scalar.dma_start(wvT[:], wvT_d.rearrange("(f p) c -> p f c", p=P))
            _trig_chunk(0, 1)
            for g in range(1, 4):
                nc.sync.dma_start(xT[:, :, g * QW:(g + 1) * QW],
                                  xT_src[:, :, g * QW:(g + 1) * QW])
            _trig_chunk(1, 3)
            nc.sync.dma_start(woT[:], woT_d.rearrange("(m p) c -> p m c", p=P))
            e_sb2 = e_sb  # keep name alive

            def emit_proj(s):
                sl = slice(s * P, (s + 1) * P)
                cs = cs_all[:, s * F:(s + 1) * F]
                sn = sn_all[:, s * F:(s + 1) * F]
                pq = ps_qk.tile([P, F], F32, tag="pq", name="pq")
                pk = ps_qk.tile([P, F], F32, tag="pk", name="pk")
                pv_ = ps_qk.tile([P, F], F32, tag="pv", name="pv")
                for f in range(FC):
                    st, sp = (f == 0), (f == FC - 1)
                    lhs = xT[:, f, sl]
                    nc.tensor.matmul(pq[:], lhs, wqT[:, f, :], start=st, stop=sp)
                    nc.tensor.matmul(pk[:], lhs, wkT[:, f, :], start=st, stop=sp)
                    nc.tensor.matmul(pv_[:], lhs, wvT[:, f, :], start=st, stop=sp)
                return s, sl, cs, sn, pq, pk, pv_, None

            def emit_tail(state):
                s, sl, cs, sn, pq, pk, pv_, qTs = state
                qcol = (s % 4) * P
                # RoPE (natural layout): out = q*cos + rot_half(q)*sin_signed
                for pp in (pq, pk):
                    t1 = rope_pool.tile([P, F], F32, tag="t1", name="t1")
                    nc.vector.tensor_mul(t1[:], pp[:], cs[:])
                    t2 = rope_pool.tile([P, F], F32, tag="t2", name="t2")
                    nc.vector.tensor_mul(_h3(t2)[:, :, 0:32],
                                         _h3(pp)[:, :, 32:64],
                                         _h3(sn)[:, :, 0:32])
                    nc.vector.tensor_mul(_h3(t2)[:, :, 32:64],
                                         _h3(pp)[:, :, 0:32],
                                         _h3(sn)[:, :, 32:64])
                    qr = rope_pool.tile([P, F], BF16, tag="qr", name="qr")
                    nc.gpsimd.tensor_add(qr[:], t1[:], t2[:])
                    pt = ring.tile([P, MC, P], BF16, tag="ring", name="pt")
                    for m in range(MC):
                        nc.tensor.transpose(pt[:, m, :],
                                            qr[:, m * P:(m + 1) * P],
                                            eye_sb[:])
                    if pp is pk:
                        nc.vector.tensor_copy(kT[:, :, sl], pt[:])
                    else:
                        nc.vector.tensor_copy(qTs[:, :, qcol:qcol + P], pt[:])
                # V with ones column per head: [V_h | 1] -> [P, NH*65]
                v3 = v_sb[s].rearrange("p (h e) -> p h e", h=NH)
                nc.vector.tensor_copy(v3[:, :, 0:64], _h3(pv_[:]))

            # ---- helpers for interleaved emission ----
            def attn_pair(qc, m, qTs, aoT, sums, inv6):
                # heads 2m (rows 0:64) and 2m+1 (rows 64:128): their K=64
                # scores matmuls use disjoint PE row groups -> issue adjacent
                # so the PE reorder window runs them concurrently.
                q0 = qc * QW
                last = 4 * qc + 3
                pvps = [ps_pv.tile([65, QW], F32, tag="pvp", name="pvp")
                        for _ in range(2)]
                exs = []
                for kc in range(last + 1):
                    k0 = kc * P
                    qlo = max(q0, k0)
                    n = q0 + QW - qlo
                    for par in range(2):
                        off = 64 * par
                        sp = ring.tile([P, QW], F32, tag="ring", name="sp")
                        nc.tensor.matmul(sp[:, 0:n],
                                         kT[off:off + 64, m, k0:k0 + P],
                                         qTs[off:off + 64, m,
                                             qlo - q0:qlo - q0 + n],
                                         start=True, stop=True)
                        ex = ex_pool.tile([P, QW], BF16, tag="ex", name="ex")
                        nc.scalar.activation(ex[:, 0:n], sp[:, 0:n],
                                             AF.Exp, scale=0.125,
                                             bias=ebias_sb[:])
                        if k0 >= q0:  # diagonal block: zero k > q
                            nc.gpsimd.tensor_mul(ex[:, 0:P], ex[:, 0:P],
                                                 tri_sb[:, par, :])
                        exs.append((par, kc, qlo, n, ex))
                for par, kc, qlo, n, ex in exs:
                    h = 2 * m + par
                    nc.tensor.matmul(pvps[par][:, qlo - q0:QW],
                                     v_sb[kc][:, h * 65:h * 65 + 65],
                                     ex[:, 0:n],
                                     start=(kc == 0), stop=(kc == last))
                for par in range(2):
                    h = 2 * m + par
                    off = 64 * par
                    if qc < 2:
                        nc.scalar.copy(aoT[m][off:off + 64, :],
                                       pvps[par][0:64, :])
                    else:
                        nc.vector.tensor_copy(aoT[m][off:off + 64, :],
                                              pvps[par][0:64, :])
                    with nc.allow_low_precision(reason="softmax sums"):
                        nc.vector.reciprocal(sums[0:1, h * QW:(h + 1) * QW],
                                             pvps[par][64:65, :])
                    if m == MC - 1 and par == 1:
                        nc.sync.dma_start(inv6[:], sums[0:1, :])

            def attn_post(qc, aoT, sums, inv6):
                q0 = qc * QW
                for m in range(MC):
                    bp = ring.tile([P, QW], F32, tag="ring", name="bp")
                    nc.tensor.matmul(bp[:], _r(e_sb[:, m * P:(m + 1) * P]),
                                     _r(inv6[:, :]), start=True, stop=True)
                    nc.vector.tensor_mul(aoT[m][:, :], aoT[m][:, :], bp[:])
                ob = ob_pool.tile([P, QW // P, HID], BF16, tag="ob", name="ob")
                for t in range(QW // P):
                    s0 = q0 + t * P
                    for half in range(2):
                        c0, c1 = half * F, half * F + F
                        fin = ring.tile([P, F], F32, tag="ring", name="fin")
                        for m in range(MC):
                            nc.tensor.matmul(fin[:],
                                             _r(aoT[m][:, s0 - q0:s0 - q0 + P]),
                                             _r(woT[:, m, c0:c1]),
                                             start=(m == 0), stop=(m == MC - 1))
                        if qc < 2:
                            nc.scalar.copy(ob[:, t, c0:c1], fin[:])
                        else:
                            nc.vector.tensor_copy(ob[:, t, c0:c1], fin[:])
                for g in range(4):
                    r0 = q0 + g * P
                    nc.sync.dma_start(
                        out_d[r0:r0 + P, :].rearrange(
                            "(t p) c -> p t c", p=P),
                        ob[:, g:g + 1, :])

            def alloc_strip(qc):
                qTs = rope_pool.tile([P, MC, QW], BF16, tag="qTs", name="qTs")
                aoT = [ao_pool.tile([P, QW], BF16, tag=f"aoTs{m}", name=f"aoTs{m}")
                       for m in range(MC)]
                sums = sums_pool.tile([1, NH * QW], F32R, tag="sums", name="sums")
                inv6 = sums_pool.tile([NH, QW], F32R, tag="inv6", name="inv6")
                return qTs, aoT, sums, inv6

            # ---- emission: group 0, then zip(attention qc, proj group qc+1) ----
            strips = {0: alloc_strip(0)}
            pending = None
            pending_post = None
            for s in range(4):
                state = emit_proj(s)[:-1] + (strips[0][0],)
                if pending is not None:
                    emit_tail(pending)
                pending = state

            for qc in range(NQ):
                qTs, aoT, sums, inv6 = strips[qc]
                if qc + 1 < NQ:
                    strips[qc + 1] = alloc_strip(qc + 1)
                    nxt = list(range(4 * qc + 4, 4 * qc + 8))
                else:
                    nxt = []
                if not nxt and pending is not None:
                    emit_tail(pending)
                    pending = None
                for mp in range(MC + 1):
                    for s_i in nxt[mp:mp + 1]:
                        state = emit_proj(s_i)[:-1] + (strips[qc + 1][0],)
                        if pending is not None:
                            emit_tail(pending)
                        pending = state
                    if mp == 1 and pending_post is not None:
                        attn_post(*pending_post)
                        pending_post = None
                    if mp < MC:
                        attn_pair(qc, mp, qTs, aoT, sums, inv6)
                pending_post = (qc, aoT, sums, inv6)
            attn_post(*pending_post)
    nc.compile()
    return nc


def _rope_tables():
    inv_freq = 1.0 / (ROPE_THETA ** (np.arange(0, D, 2, dtype=np.float32) / D))
    t = np.arange(S, dtype=np.float32)
    freqs = np.outer(t, inv_freq)                       # [S, 32]
    emb = np.concatenate([freqs, freqs], axis=-1)       # [S, 64]
    cos = np.cos(emb)
    sin = np.sin(emb)
    sin_signed = sin.copy()
    sin_signed[:, 0:32] *= -1.0                         # fold rotate_half sign
    import ml_dtypes
    cos6 = np.tile(cos, (1, NH)).astype(ml_dtypes.bfloat16)
    sin6 = np.tile(sin_signed, (1, NH)).astype(ml_dtypes.bfloat16)
    return np.ascontiguousarray(cos6), np.ascontiguousarray(sin6)


_STATE = {}


def _get_program():
    if "nc" not in _STATE:
        _STATE["nc"] = build_program()
    return _STATE["nc"]


def _make_in_maps(hidden_states, Wq, Wk, Wv, Wo):
    import ml_dtypes
    bf = ml_dtypes.bfloat16

    hs = np.asarray(hidden_states, dtype=np.float32)
    Wq = np.asarray(Wq, dtype=np.float32)
    Wk = np.asarray(Wk, dtype=np.float32)
    Wv = np.asarray(Wv, dtype=np.float32)
    Wo = np.asarray(Wo, dtype=np.float32)

    cos6, sin6 = _rope_tables()
    tri = np.broadcast_to(np.triu(np.ones((P, P), dtype=bf)),
                          (2, P, P)).transpose(1, 0, 2)  # j >= i keep
    cblob = np.concatenate([
        tri.reshape(P, 2 * P),
        np.eye(P, dtype=bf),
        np.ones((P, NH), dtype=bf),
    ], axis=1).astype(bf)
    emat = np.repeat(np.eye(NH, dtype=np.float32), D, axis=1)  # [6, 384]

    in_maps = []
    for c in range(N_CORES):
        b, g = c // 2, c % 2
        cols = slice(g * F, (g + 1) * F)
        in_maps.append({
            "xT": np.ascontiguousarray(hs[b].T).astype(bf),       # [768, S]
            "wqT": np.ascontiguousarray(Wq[cols, :].T).astype(bf),  # [768, 384]
            "wkT": np.ascontiguousarray(Wk[cols, :].T).astype(bf),
            "wvT": np.ascontiguousarray(Wv[cols, :].T).astype(bf),
            "woT": np.ascontiguousarray(Wo[:, cols].T).astype(bf),  # [384, 768]
            "cos6": cos6,
            "sin6": sin6,
            "cblob": cblob,
            "emat": emat,
        })
    return in_maps


def run(hidden_states, Wq, Wk, Wv, Wo, trace=False, **trace_kw):
    nc = _get_program()
    in_maps = _make_in_maps(hidden_states, Wq, Wk, Wv, Wo)
    res = run_bass_kernel_spmd(nc, in_maps, core_ids=list(range(N_CORES)),
                               trace=trace, **trace_kw)
    B = 4
    out = np.empty((B, S, HID), dtype=np.float32)
    for b in range(B):
        out[b] = (res.results[2 * b]["out"].astype(np.float32)
                  + res.results[2 * b + 1]["out"].astype(np.float32))
    return out, res


def kernel(hidden_states, Wq, Wk, Wv, Wo):
    out, _ = run(hidden_states, Wq, Wk, Wv, Wo,
                 trace=bool(int(os.environ.get("KERNEL_TRACE", "0"))))
    return out

